# revision 16
# baseline (speedup 1.0000x reference)
"""Trainium2 Bass kernel for nn_Alignment_42236708389244.

Dense transformer block: 1x1conv+BN shortcut, cosine-normalized MHSA over L,
m-projection, then Linear -> LayerNorm -> exact GELU.

Sharding: data-parallel over batch (B=32 -> 4 per core x 8 cores). Each core
runs an identical program on its own batch slice; no collectives.

v3 strategy (per (b, t) sequence, C=512, L=512, H=8, D=64):
  - host sends x in bf16 (conv path) AND xp = x+pe in fp8 (q/k/v path), plus
    fp8 weights wq/wk/wv (x64) and wm (x16); wout is column-centered on the
    host so LayerNorm's mean subtraction vanishes (exact), and the conv
    shortcut is scaled x1024 to match oT(x64) @ wm(x16); LN eps scales by
    1024^2 which keeps the whole FFN tail exact.
  - q/k/v/m projections and the O=vne@E matmuls run fp8 DoubleRow (2
    contraction tiles per instruction, 2x PE rate); conv/FFN/S stay bf16 for
    trunk precision.
  - q/k norms land token-part via tiny N=2 matmuls, rsqrt'd on ACT, and are
    broadcast feature-part via transpose+indicator matmuls; v norm applies
    directly with a stride-0 broadcast multiply that writes fp8 vne.
  - E = exp(S^T) stored fp8 in lk-paired tiles [128, 4, 512] so the O matmul
    can DoubleRow over lk pairs; the softmax denominator falls out of a 1/64
    ones column (which also scales oT x64 into fp8 range for the m matmul).
  - 2-stage software pipeline per bt; GELU batches every 4 bt straight from
    SBUF-resident z (2 ACT table swaps per batch, no DRAM spill).
"""

import sys

sys.path.insert(0, "/opt/trn_rl_repo")

import math
from contextlib import ExitStack

import numpy as np

import concourse.bacc as bacc
import concourse.bass as bass
import concourse.tile as tile
from concourse import mybir
from concourse.bass_utils import run_bass_kernel_spmd

B, C, T, L, H, D, OU = 32, 512, 4, 512, 8, 64, 512
NCORES = 8
BLOC = B // NCORES
NBT = BLOC * T
EPS = 1e-5
WS = 64.0  # fp8 scale on wq/wk/wv
MS = 16.0  # fp8 scale on wm
OS = 64.0  # oT scale (via 1/64 ones column)
CS = OS * MS  # = 1024, scale of the os_t trunk
F32 = mybir.dt.float32
BF16 = mybir.dt.bfloat16
FP8 = mybir.dt.float8e4
AF = mybir.ActivationFunctionType
ALU = mybir.AluOpType
DR = mybir.MatmulPerfMode.DoubleRow

_cache = {}


def _build(qkv_bias: bool = False, ln_id: bool = True):
    nc = bacc.Bacc(trn_type="TRN2", target_bir_lowering=False, debug=False)

    x_d = nc.dram_tensor("x", [BLOC, C, T, L], BF16, kind="ExternalInput")
    xp_d = nc.dram_tensor("xp", [BLOC, C, T, L], FP8, kind="ExternalInput")
    out_d = nc.dram_tensor("out", [BLOC, OU, T, L], F32, kind="ExternalOutput")
    w8_names = ["wq", "wk", "wv", "wm"]
    w8_d = {n: nc.dram_tensor(n, [C, C], FP8, kind="ExternalInput") for n in w8_names}
    wct_d = nc.dram_tensor("wct", [C, C], BF16, kind="ExternalInput")
    wout_d = nc.dram_tensor("wout", [C, OU], BF16, kind="ExternalInput")
    qkvb_d = nc.dram_tensor("qkvb", [1, 3 * C], F32, kind="ExternalInput")
    onesr_d = nc.dram_tensor("onesrd", [1, 128], F32, kind="ExternalInput")
    # packed per-partition columns:
    # [bias_sc*1024, b_out_c, ln_g, ln_b, bq*64(4), bk*64(4)] -> [128, 24]
    cols_d = nc.dram_tensor("cols", [128, 24], F32, kind="ExternalInput")
    identbfd = nc.dram_tensor("identbfd", [128, 128], BF16, kind="ExternalInput")
    inv512d = nc.dram_tensor("inv512d", [128, 128], BF16, kind="ExternalInput")
    bd2d = nc.dram_tensor("bd2d", [128, 2], BF16, kind="ExternalInput")
    ind8d = nc.dram_tensor("ind8d", [8, 4 * 128], BF16, kind="ExternalInput")

    with tile.TileContext(nc) as tc, ExitStack() as ctx:
        con = ctx.enter_context(tc.tile_pool(name="con", bufs=1))
        wrk = ctx.enter_context(tc.tile_pool(name="wrk", bufs=1))
        pp = ctx.enter_context(tc.tile_pool(name="pp", bufs=1, space="PSUM"))

        # ---- constants ----
        ident_bf = con.tile([128, 128], BF16, tag="ident_bf")
        nc.sync.dma_start(out=ident_bf, in_=identbfd.ap())
        inv512 = con.tile([128, 128], BF16, tag="inv512")
        nc.sync.dma_start(out=inv512, in_=inv512d.ap())
        bd2 = con.tile([128, 2], BF16, tag="bd2")
        nc.sync.dma_start(out=bd2, in_=bd2d.ap())
        ind8 = con.tile([8, 4 * 128], BF16, tag="ind8")
        nc.sync.dma_start(out=ind8, in_=ind8d.ap())
        ln8h_c = con.tile([128, 1], F32, tag="ln8h_c")
        nc.vector.memset(ln8h_c, math.log(0.125))  # full 1/8 folded into k scale
        mln64_c = con.tile([128, 1], F32, tag="mln64_c")
        nc.vector.memset(mln64_c, -math.log(OS))
        eps_c = con.tile([128, 1], F32, tag="eps_c")
        nc.vector.memset(eps_c, EPS * CS * CS)
        cols = con.tile([128, 24], F32, tag="cols")
        nc.sync.dma_start(out=cols, in_=cols_d.ap())
        bias_sc = cols[:, 0:4]
        b_out_c = cols[:, 4:8]
        ln_g_c = cols[:, 8:12]
        ln_b_c = cols[:, 12:16]
        bq_c = cols[:, 16:20]
        bk_c = cols[:, 20:24]

        wt8 = {}
        for n in w8_names:
            wt8[n] = con.tile([128, 4, C], FP8, tag=n, name=f"wt_{n}")
            nc.sync.dma_start(
                out=wt8[n], in_=w8_d[n].ap().rearrange("(cc p) n -> p cc n", p=128)
            )
        wct = con.tile([128, 4, C], BF16, tag="wct")
        nc.sync.dma_start(
            out=wct, in_=wct_d.ap().rearrange("(cc p) n -> p cc n", p=128)
        )
        wout = con.tile([128, 4, C], BF16, tag="wout")
        nc.sync.dma_start(
            out=wout, in_=wout_d.ap().rearrange("(cc p) n -> p cc n", p=128)
        )
        if qkv_bias:
            qkvb = con.tile([1, 3 * C], F32, tag="qkvb")
            nc.sync.dma_start(out=qkvb, in_=qkvb_d.ap())
            onesr = con.tile([1, 128], F32, tag="onesr")
            nc.sync.dma_start(out=onesr, in_=onesr_d.ap())

        state = [None, None]

        def emit_load(st_, bt):
            b, t = bt // T, bt % T
            xb = wrk.tile([128, 4, L], BF16, tag="xb", bufs=3, name="xb")
            nc.sync.dma_start(
                out=xb, in_=x_d.ap()[b, :, t, :].rearrange("(cc p) l -> p cc l", p=128)
            )
            xpb = wrk.tile([128, 4, L], FP8, tag="xpb", bufs=3, name="xpb")
            nc.sync.dma_start(
                out=xpb,
                in_=xp_d.ap()[b, :, t, :].rearrange("(cc p) l -> p cc l", p=128),
            )
            st_["xb"] = xb
            st_["xpb"] = xpb
            st_["oT"] = wrk.tile([128, 4, L], FP8, tag="oT", bufs=2, name="oT")
            st_["eA"] = [None] * 4
            st_["eB"] = [None] * 4

        def emit_conv(st_):
            xb = st_["xb"]
            os_t = wrk.tile([128, 4, L], BF16, tag="os", bufs=3, name="os_t")
            for oc in range(4):
                c_ps = pp.tile([128, 512], F32, tag="mm", bufs=3, name="c_ps")
                for cc in range(4):
                    nc.tensor.matmul(
                        c_ps,
                        wct[:, cc, oc * 128 : (oc + 1) * 128],
                        xb[:, cc, :],
                        start=(cc == 0),
                        stop=(cc == 3),
                    )
                nc.vector.tensor_scalar(
                    out=os_t[:, oc, :], in0=c_ps, scalar1=CS,
                    scalar2=bias_sc[:, oc : oc + 1], op0=ALU.mult, op1=ALU.add,
                )
            st_["os"] = os_t

        def emit_proj(st_, which):
            xpb = st_["xpb"]
            wname, b_col, tag = (
                ("wq", bq_c, "pqb") if which == "q" else ("wk", bk_c, "pkb")
            )
            pb = wrk.tile([128, 4, 512], BF16, tag=tag, bufs=2, name=tag)
            sq_t = wrk.tile([128, 4, 512], BF16, tag="sqq", bufs=3, name="sq_t")
            for oc in range(4):
                p_ps = pp.tile([128, 512], F32, tag="mm", bufs=3, name="p_ps")
                for i in range(2):
                    nc.tensor.matmul(
                        p_ps,
                        wt8[wname][:, 2 * i : 2 * i + 2, oc * 128 : (oc + 1) * 128],
                        xpb[:, 2 * i : 2 * i + 2, :],
                        start=(i == 0),
                        stop=(i == 1),
                        perf_mode=DR,
                    )
                if qkv_bias:
                    nc.vector.tensor_scalar(
                        out=pb[:, oc, :],
                        in0=p_ps,
                        scalar1=b_col[:, oc : oc + 1],
                        scalar2=None,
                        op0=ALU.add,
                    )
                else:
                    nc.vector.tensor_scalar(
                        out=pb[:, oc, :], in0=p_ps, scalar1=0.0, scalar2=None,
                        op0=ALU.add,
                    )
                nc.vector.tensor_mul(
                    out=sq_t[:, oc, :], in0=pb[:, oc, :], in1=pb[:, oc, :]
                )
            st_[which + "T"] = pb
            st_[which + "sq"] = sq_t

        def _sumsq_mm(sq_t):
            n_ps = pp.tile([128, 512], F32, tag="mm", bufs=3, name="n_ps")
            nview = n_ps[:, 0:32].rearrange("p (lc h) -> p lc h", h=8)
            for oc in range(4):
                for lc in range(4):
                    nc.tensor.matmul(
                        nview[:, lc, 2 * oc : 2 * oc + 2],
                        sq_t[:, oc, lc * 128 : (lc + 1) * 128],
                        bd2,
                        start=True,
                        stop=True,
                    )
            return n_ps

        def emit_norms(st_, which):
            # q: per-head sumsq via bd2-as-weights (head-part [2,512] outs at
            # partition offsets 0/32/64/96), compacted by one DMA, rsqrt'd on
            # ACT, broadcast feature-part via indicator matmuls.
            pb = st_[which + "T"]
            sq_t = st_[which + "sq"]
            n8_ps = pp.tile([128, 512], F32, tag="mm", bufs=3, name="n8_ps")
            for oc in range(4):
                nc.tensor.matmul(
                    n8_ps[32 * oc : 32 * oc + 2, :],
                    bd2,
                    sq_t[:, oc, :],
                    start=True,
                    stop=True,
                    tile_position=(0, 32 * oc),
                )
            n8s = wrk.tile([128, 512], BF16, tag="n8s", bufs=2, name="n8s")
            nc.vector.tensor_scalar(
                out=n8s, in0=n8_ps, scalar1=0.0, scalar2=None, op0=ALU.add
            )
            r8c = wrk.tile([8, 512], BF16, tag="r8c", bufs=2, name="r8c")
            for oc in range(4):
                nc.sync.dma_start(
                    out=r8c[2 * oc : 2 * oc + 2, :],
                    in_=n8s[32 * oc : 32 * oc + 2, :],
                )
            r8f = wrk.tile([8, 512], F32, tag="r8f", bufs=2, name="r8f")
            r8 = wrk.tile([8, 512], BF16, tag="r8", bufs=2, name="r8")
            nc.scalar.activation(r8f, r8c, AF.Ln)
            nc.scalar.activation(r8, r8f, AF.Exp, scale=-0.5)
            for oc in range(4):
                rf_ps = pp.tile([128, 512], F32, tag="mm", bufs=3, name="rf_ps")
                nc.tensor.matmul(
                    rf_ps,
                    ind8[:, oc * 128 : (oc + 1) * 128],
                    r8,
                    start=True,
                    stop=True,
                )
                nc.vector.tensor_mul(out=pb[:, oc, :], in0=pb[:, oc, :], in1=rf_ps)

        def emit_knorm(st_):
            # k: rsqrt (x 1/8) kept token-part; applied as per-partition scale
            # in the softmax exp
            n_ps = _sumsq_mm(st_["ksq"])
            rt = wrk.tile([128, 32], F32, tag="rt", bufs=2, name="rt")
            rtk = wrk.tile([128, 32], F32, tag="rtk", bufs=2, name="rtk")
            nc.scalar.activation(rt, n_ps[:, 0:32], AF.Ln)
            nc.scalar.activation(rtk, rt, AF.Exp, bias=ln8h_c[:], scale=-0.5)
            st_["rtk"] = rtk

        def emit_v(st_):
            xpb = st_["xpb"]
            vne = wrk.tile([128, 4, 8 * 66], FP8, tag="vne", bufs=3, name="vne")
            vne_v = vne.rearrange("p lc (h e) -> p lc h e", e=66)
            nc.vector.memset(vne_v[:, :, :, 65:66], 0.0)
            sqv = wrk.tile([128, 4, 512], BF16, tag="sqq", bufs=3, name="sqv")
            for lc in range(4):
                v_ps = pp.tile([128, 512], F32, tag="mm", bufs=3, name="v_ps")
                if qkv_bias:
                    nc.tensor.matmul(
                        v_ps,
                        onesr[0:1, 0:128],
                        qkvb[0:1, 2 * C : 3 * C],
                        start=True,
                        stop=False,
                    )
                for i in range(2):
                    nc.tensor.matmul(
                        v_ps,
                        xpb[:, 2 * i : 2 * i + 2, lc * 128 : (lc + 1) * 128],
                        wt8["wv"][:, 2 * i : 2 * i + 2, :],
                        start=(i == 0 and not qkv_bias),
                        stop=(i == 1),
                        perf_mode=DR,
                    )
                nc.vector.tensor_scalar(
                    out=vne_v[:, lc, :, 0:64],
                    in0=v_ps,
                    scalar1=0.0,
                    scalar2=None,
                    op0=ALU.add,
                )
                nc.vector.tensor_mul(
                    out=sqv[:, lc, :],
                    in0=vne_v[:, lc, :, 0:64],
                    in1=vne_v[:, lc, :, 0:64],
                )
            nv = wrk.tile([128, 32], F32, tag="nv", bufs=2, name="nv")
            nc.vector.tensor_reduce(
                nv,
                sqv.rearrange("p lc (h d) -> p (lc h) d", d=D),
                axis=mybir.AxisListType.X,
                op=ALU.add,
            )
            nc.scalar.activation(nv, nv, AF.Ln)  # nv = ln(sv)
            lnrv = wrk.tile([128, 32], F32, tag="lnrv", bufs=2, name="lnrv")
            nc.vector.tensor_scalar(
                out=lnrv, in0=nv, scalar1=-0.5, scalar2=None, op0=ALU.mult
            )
            onec = wrk.tile([128, 32], F32, tag="onec", bufs=2, name="onec")
            nc.scalar.activation(onec, nv, AF.Exp, bias=mln64_c[:], scale=0.5)
            nc.vector.tensor_scalar(
                out=vne_v[:, :, :, 64:65],
                in0=onec,
                scalar1=0.0,
                scalar2=None,
                op0=ALU.add,
            )
            st_["vne"] = vne
            st_["lnrv"] = lnrv
            st_["onec"] = onec

        def emit_S_lk(st_, pair, lk):
            knT, qnT = st_["kT"], st_["qT"]
            stA = pp.tile([128, 512], F32, tag="st", bufs=3, name="stA")
            stB = pp.tile([128, 512], F32, tag="st", bufs=3, name="stB")
            nc.tensor.matmul(
                stA,
                knT[0:64, pair, lk * 128 : (lk + 1) * 128],
                qnT[0:64, pair, :],
                start=True,
                stop=True,
            )
            nc.tensor.matmul(
                stB,
                knT[64:128, pair, lk * 128 : (lk + 1) * 128],
                qnT[64:128, pair, :],
                start=True,
                stop=True,
            )
            if lk == 0:
                st_["eA"][pair] = wrk.tile(
                    [128, 4, 512], FP8, tag="eab", bufs=17, name="eA"
                )
                st_["eB"][pair] = wrk.tile(
                    [128, 4, 512], FP8, tag="eab", bufs=17, name="eB"
                )
            rtk, lnrv = st_["rtk"], st_["lnrv"]
            cA = lk * 8 + 2 * pair
            nc.scalar.activation(
                st_["eA"][pair][:, lk, :], stA, AF.Exp,
                bias=lnrv[:, cA : cA + 1], scale=rtk[:, cA : cA + 1],
            )
            nc.scalar.activation(
                st_["eB"][pair][:, lk, :], stB, AF.Exp,
                bias=lnrv[:, cA + 1 : cA + 2], scale=rtk[:, cA + 1 : cA + 2],
            )

        def emit_O(st_, pair):
            vne, oT = st_["vne"], st_["oT"]
            ozts = [
                pp.tile([128, 512], F32, tag="oz", bufs=2, name="ozt")
                for _ in range(2)
            ]
            for half in range(2):
                h = 2 * pair + half
                eX = st_["eA"][pair] if half == 0 else st_["eB"][pair]
                for i in range(2):
                    nc.tensor.matmul(
                        ozts[half][0:66, :],
                        vne[:, 2 * i : 2 * i + 2, h * 66 : (h + 1) * 66],
                        eX[:, 2 * i : 2 * i + 2, :],
                        start=(i == 0),
                        stop=(i == 1),
                        perf_mode=DR,
                    )
            for half in range(2):
                ozt = ozts[half]
                oz = ozt[0:65, :]
                zrec = wrk.tile([65, 512], F32, tag="zr", bufs=2, name="zrec")
                nc.vector.reciprocal_approx_fast(out=zrec, in_=oz)
                row = zrec[64:65, :]
                bsrc = bass.AP(
                    tensor=row.tensor,
                    offset=row.offset,
                    ap=[list(row.ap[0]), [0, 64]] + [list(a) for a in row.ap[1:]],
                )
                zb = wrk.tile([64, 512], F32, tag="zb", bufs=2, name="zb")
                nc.sync.dma_start(out=zb, in_=bsrc)
                if half == 0:
                    nc.vector.tensor_mul(
                        out=oT[0:64, pair, :], in0=oz[0:64, :], in1=zb
                    )
                else:
                    otb = wrk.tile([64, 512], FP8, tag="otb", bufs=1, name="otb")
                    nc.vector.tensor_mul(out=otb, in0=oz[0:64, :], in1=zb)
                    nc.sync.dma_start(out=oT[64:128, pair, :], in_=otb)

        def emit_m(st_):
            os_t, oT = st_["os"], st_["oT"]
            for oc in range(4):
                m_ps = pp.tile([128, 512], F32, tag="mm", bufs=3, name="m_ps")
                for i in range(2):
                    nc.tensor.matmul(
                        m_ps,
                        wt8["wm"][:, 2 * i : 2 * i + 2, oc * 128 : (oc + 1) * 128],
                        oT[:, 2 * i : 2 * i + 2, :],
                        start=(i == 0),
                        stop=(i == 1),
                        perf_mode=DR,
                    )
                nc.vector.tensor_add(
                    out=os_t[:, oc, :], in0=m_ps, in1=os_t[:, oc, :]
                )

        def emit_ffn(st_):
            os_t = st_["os"]
            z_t = wrk.tile([128, 4, L], BF16, tag="zz", bufs=6, name="z_t")
            for oc in range(4):
                f_ps = pp.tile([128, 512], F32, tag="mm", bufs=3, name="f_ps")
                for cc in range(4):
                    nc.tensor.matmul(
                        f_ps,
                        wout[:, cc, oc * 128 : (oc + 1) * 128],
                        os_t[:, cc, :],
                        start=(cc == 0),
                        stop=(cc == 3),
                    )
                nc.vector.tensor_scalar(
                    out=z_t[:, oc, :], in0=f_ps,
                    scalar1=b_out_c[:, oc : oc + 1], scalar2=None, op0=ALU.add,
                )
            st_["zt"] = z_t

        def emit_ln(st_):
            z_t = st_["zt"]
            sqz = wrk.tile([128, 4, L], BF16, tag="sqz", bufs=1, name="sqz")
            for oc in range(4):
                nc.gpsimd.tensor_mul(
                    out=sqz[:, oc, :], in0=z_t[:, oc, :], in1=z_t[:, oc, :]
                )
            var_ps = pp.tile([128, 512], F32, tag="mm", bufs=3, name="var_ps")
            for oc in range(4):
                nc.tensor.matmul(
                    var_ps, inv512, sqz[:, oc, :], start=(oc == 0), stop=(oc == 3)
                )
            rstd = wrk.tile([128, 512], F32, tag="rstd", bufs=2, name="rstd")
            nc.scalar.activation(rstd, var_ps, AF.Ln, bias=eps_c[:])
            nc.scalar.activation(rstd, rstd, AF.Exp, scale=-0.5)
            st_["rstd"] = rstd
            for oc in range(4):
                nc.gpsimd.tensor_mul(out=z_t[:, oc, :], in0=z_t[:, oc, :], in1=rstd)
                if not ln_id:
                    nc.gpsimd.tensor_scalar(
                        out=z_t[:, oc, :],
                        in0=z_t[:, oc, :],
                        scalar1=ln_g_c[:, oc : oc + 1],
                        scalar2=ln_b_c[:, oc : oc + 1],
                        op0=ALU.mult,
                        op1=ALU.add,
                    )

        def emit_gelu(z_t, bt, gate):
            b, t = bt // T, bt % T
            for oc in range(4):
                gout = wrk.tile([128, L], F32, tag="gout", bufs=8, name="gout")
                if gate is not None:
                    nc.scalar.activation(gout, z_t[:, oc, :], AF.Gelu, scale=gate[:])
                else:
                    nc.scalar.activation(gout, z_t[:, oc, :], AF.Gelu)
                nc.gpsimd.dma_start(
                    out=out_d.ap()[b, oc * 128 : (oc + 1) * 128, t, :], in_=gout
                )

        # ---- main loop: interleave front(bt) groups with back(bt-1) ----
        zhist = []  # (z_tile, bt) awaiting gelu

        def flush_gelu(cur, prev):
            # Gate the gelu batch on the last exp-table users emitted before
            # this point so the scheduler can't interleave them (each
            # interleave costs two 1.3us ACT table loads).
            gate = None
            if cur is not None:
                g1 = wrk.tile([128, 1], F32, tag="g1", bufs=2, name="g1")
                g2 = wrk.tile([128, 1], F32, tag="g2", bufs=2, name="g2")
                gate = wrk.tile([128, 1], F32, tag="gate", bufs=2, name="gate")
                nc.vector.scalar_tensor_tensor(
                    out=g1, in0=cur["qT"][:, 0, 0:1], scalar=0.0,
                    in1=cur["rtk"][:, 0:1], op0=ALU.mult, op1=ALU.mult,
                )
                nc.vector.scalar_tensor_tensor(
                    out=g2, in0=cur["onec"][:, 0:1], scalar=0.0,
                    in1=prev["rstd"][:, 0:1], op0=ALU.mult, op1=ALU.mult,
                )
                nc.vector.scalar_tensor_tensor(
                    out=gate, in0=g1, scalar=1.0, in1=g2,
                    op0=ALU.add, op1=ALU.subtract,
                )
            while zhist:
                zt_, bt_ = zhist.pop(0)
                emit_gelu(zt_, bt_, gate)

        # 3-stage pipeline: iteration i runs front(bt=i), S+exp(bt=i-1),
        # and O/m/ffn/ln(bt=i-2) so every engine sees three independent
        # dependency streams.
        states = {}
        for it in range(NBT + 2):
            bt = it
            cur = None
            if bt < NBT:
                cur = {}
                states[bt] = cur
                emit_load(cur, bt)
            sS = states.get(it - 1)
            sO = states.get(it - 2)
            if sS:
                for lk in range(4):
                    emit_S_lk(sS, 0, lk)
            if cur:
                emit_conv(cur)
            if sO:
                emit_O(sO, 0)
                emit_O(sO, 1)
            if cur:
                emit_proj(cur, "q")
            if sS:
                for lk in range(4):
                    emit_S_lk(sS, 1, lk)
            if sO:
                emit_O(sO, 2)
                emit_O(sO, 3)
            if cur:
                emit_norms(cur, "q")
            if sO:
                emit_m(sO)
            if cur:
                emit_proj(cur, "k")
            if sS:
                for lk in range(4):
                    emit_S_lk(sS, 2, lk)
            if cur:
                emit_knorm(cur)
            if sO:
                emit_ffn(sO)
            if cur:
                emit_v(cur)
            if sS:
                for lk in range(4):
                    emit_S_lk(sS, 3, lk)
            if sO:
                emit_ln(sO)
                zhist.append((sO["zt"], it - 2))
                if len(zhist) >= 4 and cur is not None:
                    flush_gelu(cur, sO)
            if it - 3 >= 0:
                states.pop(it - 3, None)
        flush_gelu(None, None)

    keep = {"natural_log_exp_and_others", "gelu_and_others"}
    orig_tables = bacc.get_activation_tables

    def patched_tables(arch):
        return {
            name: (funcs if name in keep else set())
            for name, funcs in orig_tables(arch).items()
        }

    bacc.get_activation_tables = patched_tables
    try:
        nc.finalize()
    finally:
        bacc.get_activation_tables = orig_tables
    return nc


def _prep(inputs):
    f = np.float32
    x = np.asarray(inputs["x"], f)
    pe = np.asarray(inputs["pe"], f)
    w_q, b_q = np.asarray(inputs["w_q"], f), np.asarray(inputs["b_q"], f)
    w_kv, b_kv = np.asarray(inputs["w_kv"], f), np.asarray(inputs["b_kv"], f)
    w_m, b_m = np.asarray(inputs["w_m"], f), np.asarray(inputs["b_m"], f)
    conv_w, conv_b = np.asarray(inputs["conv_w"], f), np.asarray(inputs["conv_b"], f)
    bn_g, bn_b = np.asarray(inputs["bn_gamma"], f), np.asarray(inputs["bn_beta"], f)
    bn_m, bn_v = np.asarray(inputs["bn_mean"], f), np.asarray(inputs["bn_var"], f)
    w_out, b_out = np.asarray(inputs["w_out"], f), np.asarray(inputs["b_out"], f)
    ln_g, ln_b = np.asarray(inputs["ln_g"], f), np.asarray(inputs["ln_b"], f)

    scale = bn_g / np.sqrt(bn_v + EPS)
    wct = np.ascontiguousarray((conv_w * scale[:, None]).T)  # [C, O]
    bias_sc = ((conv_b - bn_m) * scale + bn_b + b_m) * CS  # b_m folded + trunk scale

    # center wout columns (makes LayerNorm mean subtraction exact and free)
    w_outc = w_out - w_out.mean(axis=1, keepdims=True)
    b_outc = b_out - b_out.mean()

    w_k, w_v = w_kv[:, :C], w_kv[:, C:]
    b_k, b_v = b_kv[:C], b_kv[C:]
    import ml_dtypes

    bf = ml_dtypes.bfloat16
    f8 = ml_dtypes.float8_e4m3fn

    def col(v):
        return np.ascontiguousarray(v.reshape(4, 128).T)  # [128, 4]

    cols = np.concatenate(
        [
            col(bias_sc),
            col(b_outc),
            col(ln_g),
            col(ln_b),
            col(b_q * WS),
            col(b_k * WS),
        ],
        axis=1,
    )

    ind8 = np.zeros((8, 4 * 128), dtype=bf)
    for oc in range(4):
        ind8[2 * oc, oc * 128 : oc * 128 + 64] = 1
        ind8[2 * oc + 1, oc * 128 + 64 : oc * 128 + 128] = 1
    bd2 = np.zeros((128, 2), dtype=bf)
    bd2[0:64, 0] = 1
    bd2[64:128, 1] = 1

    shared = {
        "identbfd": np.eye(128, dtype=bf),
        "inv512d": np.full((128, 128), 1.0 / 512.0, dtype=bf),
        "bd2d": bd2,
        "ind8d": ind8,
        "wq": np.ascontiguousarray((w_q * WS).astype(f8)),
        "wk": np.ascontiguousarray((w_k * WS).astype(f8)),
        "wv": np.ascontiguousarray((w_v * WS).astype(f8)),
        "wm": np.ascontiguousarray((w_m * MS).astype(f8)),
        "wct": np.ascontiguousarray(wct.astype(bf)),
        "wout": np.ascontiguousarray(w_outc.astype(bf)),
        "qkvb": np.concatenate([b_q, b_k, b_v * WS]).reshape(1, 3 * C).astype(f),
        "onesrd": np.ones((1, 128), dtype=f),
        "cols": np.ascontiguousarray(cols),
    }
    # xp = x + pe, in fp8, same [b, c, t, l] layout as x
    xp = x + pe.T[None, :, None, :]
    in_maps = []
    for core in range(NCORES):
        m = dict(shared)
        m["x"] = np.ascontiguousarray(x[core * BLOC : (core + 1) * BLOC].astype(bf))
        m["xp"] = np.ascontiguousarray(
            xp[core * BLOC : (core + 1) * BLOC].astype(f8)
        )
        in_maps.append(m)
    return in_maps


def kernel(**inputs) -> np.ndarray:
    qb = bool(
        np.any(np.asarray(inputs["b_q"])) or np.any(np.asarray(inputs["b_kv"]))
    )
    ln_id = bool(
        np.all(np.asarray(inputs["ln_g"]) == 1.0)
        and np.all(np.asarray(inputs["ln_b"]) == 0.0)
    )
    key = ("nc", qb, ln_id)
    if key not in _cache:
        _cache[key] = _build(qkv_bias=qb, ln_id=ln_id)
    nc = _cache[key]
    in_maps = _prep(inputs)
    res = run_bass_kernel_spmd(nc, in_maps, core_ids=list(range(NCORES)))
    return np.concatenate([r["out"] for r in res.results], axis=0)


if __name__ == "__main__":
    nc = _build()
    print("build ok; instructions:", len(nc.inst_map))
